# revision 6
# baseline (speedup 1.0000x reference)
"""Trainium2 Bass kernel for a KAN (Kolmogorov-Arnold) layer.

Computation (see reference):
  out = silu(x) @ base_weight.T + bspline_basis(x).reshape(B,-1) @ (spline_weight*scaler).reshape(O,-1).T

Key ideas:
  * Data-parallel: batch 4096 is split across 8 NeuronCores (512 rows each);
    weights are replicated. No inter-core communication.
  * The cubic B-spline basis over the uniform grid (knots -2.2 + 0.4*j) has a
    closed form per output channel c (c = 0..7):
        s = (x + 2.2) / 0.4,   v = 2 - |s - (c+2)|
        6 * basis_c = relu(v)^3 - 4 * relu(v-1)^3
    (truncated-power representation of the cardinal cubic B-spline; the 1/6 is
    folded into the spline weights on the host).
  * Both matmuls run in bf16 on the tensor engine, accumulating fp32 into the
    same PSUM tiles: out[b,o] = sum_k silu_T[k,b] * WbT[k,o]  (k = 1024)
                              + sum_k  d_T[k,b]  * W2T[k,o]   (k = 8192)
    with k (contraction) on partitions, batch on PSUM partitions.
  * Per-core layouts are prepared on the host so every DMA is contiguous.
"""

import numpy as np
import ml_dtypes

N_CORES = 8
B_FULL = 4096
B_SH = B_FULL // N_CORES  # 512
IN_F = 1024
OUT_F = 1024
N_COEF = 8
GRID_T0 = -2.2  # first knot
GRID_H = 0.4    # knot spacing
C4 = 4.0 ** (1.0 / 3.0)

_CACHE = {}


def _build_program():
    import concourse.bass as bass
    import concourse.tile as tile
    from concourse import mybir
    from concourse.vector_clock import ScopedClock

    f32 = mybir.dt.float32
    bf16 = mybir.dt.bfloat16
    AF = mybir.ActivationFunctionType
    ALU = mybir.AluOpType

    class SplitDrainTileContext(tile.TileContext):
        """The pinned walrus build only accepts a single sem-wait per
        instruction; hoist excess waits onto injected same-engine NoOps
        placed immediately before the over-subscribed instruction."""

        def _split_excess_waits(self):
            nc = self.nc
            k = 0
            for func in nc.m.functions:
                for bb in func.blocks:
                    il = bb.instructions
                    i = 0
                    while i < len(il):
                        inst = il[i]
                        si = inst.sync_info
                        if si is not None and si.on_wait and len(si.on_wait) > 1:
                            extra = list(si.on_wait)[1:]
                            del si.on_wait[1:]
                            for w in extra:
                                nop = mybir.InstNoOp(
                                    name=f"wsplit-{k}",
                                    engine=inst.engine,
                                    bass_nofuse=True,
                                    sync_info=mybir.SyncInfo(
                                        on_wait=[w], on_update=[]),
                                )
                                k += 1
                                nc.register_instruction(nop)
                                il.insert(i, nop)
                                i += 1
                        i += 1

        def _drain_and_barrier(self, tick_clock, wait_clock):
            nc = self.nc
            drain_inst = nc.sync.drain()
            wait_clock.add_sem_waits(
                drain_inst.ins, ScopedClock({None: tick_clock.global_clock})
            )
            self._split_excess_waits()
            nc.all_engine_barrier()
            assert self.sems is not None
            popped = nc._tile_sem_poison_stack.pop()
            assert popped is self._sem_poison
            nc.clear_and_free_semaphores(list(self.sems.allocated().values()))
            nc.all_engine_barrier()

    nc = bass.Bass("TRN2", target_bir_lowering=False, debug=False,
                   num_devices=N_CORES)

    # Activation float biases must exist as const APs (only 0.0/1.0 built in).
    bias_vals = [2.0, C4] + [3.5 - c for c in range(8)]
    for val in bias_vals:
        t = nc.alloc_sbuf_tensor(f"const-float32-{val}", [128, 1],
                                 mybir.dt.float32)
        nc.gpsimd.memset(t.ap(), val)
        nc.const_aps.aps[(mybir.dt.float32, val)] = t.ap()
    nc.all_engine_barrier()

    # Host-prepared layouts (per core):
    #  xt [128, 4096] f32 : xt[p, t*512+b] = x_shard[b, t*128+p]
    #  wb [128, 8192] bf16: wb[p, t*1024+o] = base_weight[o, t*128+p]
    #  w2 [128, 65536] bf16: w2[p, (c*8+t)*1024+o] = eff_w[o, t*128+p, c]/6
    xt_ap = nc.dram_tensor("xt", [128, 8 * B_SH], f32, kind="ExternalInput").ap()
    wb_ap = nc.dram_tensor("wb", [128, 8 * 1024], bf16, kind="ExternalInput").ap()
    w2_ap = nc.dram_tensor("w2", [128, 64 * 1024], bf16, kind="ExternalInput").ap()
    out_ap = nc.dram_tensor("out", [B_SH, OUT_F], f32, kind="ExternalOutput").ap()

    HW = 4 * B_SH  # 2048: elementwise chunk width (half of the 4096 free dim)

    with SplitDrainTileContext(nc) as tc:
        import contextlib
        ctx = contextlib.ExitStack()
        with ctx:
            io_pool = ctx.enter_context(tc.tile_pool(name="io", bufs=1))
            wpool = ctx.enter_context(tc.tile_pool(name="w", bufs=6))
            apool = ctx.enter_context(tc.tile_pool(name="a", bufs=2))
            tpool = ctx.enter_context(tc.tile_pool(name="t", bufs=2))
            dpool = ctx.enter_context(tc.tile_pool(name="d", bufs=6))
            opool = ctx.enter_context(tc.tile_pool(name="o", bufs=4))
            psum_pool = ctx.enter_context(
                tc.tile_pool(name="ps", bufs=1, space="PSUM"))

            # ---- prologue: load x, compute s and silu ----
            xt = io_pool.tile([128, 8 * B_SH], f32, name="xt", tag="xt")
            nc.sync.dma_start(xt[:], xt_ap[:])

            silu_t = io_pool.tile([128, 8 * B_SH], bf16, name="silu", tag="silu")
            nc.scalar.activation(silu_t[:], xt[:], AF.Silu)

            # ---- PSUM output tiles: (bt, oc) -> [128 b, 512 o] ----
            psum = {}
            for bt in range(4):
                for oc in range(2):
                    psum[(bt, oc)] = psum_pool.tile([128, 512], f32, name=f"ps{bt}{oc}", tag=f"ps{bt}{oc}")

            def mm_block(lhs_tile, lhs_col0, w_tile, w_col0, start, stop):
                # 8 matmuls: 4 batch tiles x 2 out chunks, one K-tile (128)
                for bt in range(4):
                    for oc in range(2):
                        nc.tensor.matmul(
                            psum[(bt, oc)][:, :],
                            lhs_tile[:, lhs_col0 + bt * 128:
                                     lhs_col0 + bt * 128 + 128],
                            w_tile[:, w_col0 + oc * 512: w_col0 + oc * 512 + 512],
                            start=start, stop=stop,
                        )

            # ---- base matmuls: 8 K-tiles over in_features ----
            for j in range(4):  # wb DMA'd as 4 x [128, 2048] (2 K-tiles each)
                wt = wpool.tile([128, 2048], bf16, name="w", tag="w")
                nc.sync.dma_start(wt[:], wb_ap[:, j * 2048:(j + 1) * 2048])
                for tt in range(2):
                    t = 2 * j + tt
                    mm_block(silu_t, t * B_SH, wt, tt * 1024,
                             start=(t == 0), stop=False)

            # ---- spline channels ----
            for c in range(8):
                dhalves = []
                for h in range(2):  # elementwise in 2 chunks of [128, 2048]
                    sl = slice(h * HW, (h + 1) * HW)
                    a = apool.tile([128, HW], f32, name="a", tag="a")
                    nc.scalar.activation(a[:], xt[:, sl], AF.Abs,
                                         bias=3.5 - c, scale=1.0 / GRID_H)
                    r1 = tpool.tile([128, HW], bf16, name="r1", tag="r1")
                    nc.scalar.activation(r1[:], a[:], AF.Relu, bias=2.0,
                                         scale=-1.0)
                    r2 = tpool.tile([128, HW], bf16, name="r2", tag="r2")
                    nc.scalar.activation(r2[:], a[:], AF.Relu, bias=C4,
                                         scale=-C4)
                    s1 = tpool.tile([128, HW], bf16, name="s1", tag="s1")
                    nc.vector.tensor_mul(s1[:], r1[:], r1[:])
                    s2 = tpool.tile([128, HW], bf16, name="s2", tag="s2")
                    nc.vector.tensor_mul(s2[:], r2[:], r2[:])
                    c1 = tpool.tile([128, HW], bf16, name="c1", tag="c1")
                    nc.vector.tensor_mul(c1[:], s1[:], r1[:])
                    c2 = tpool.tile([128, HW], bf16, name="c2", tag="c2")
                    nc.vector.tensor_mul(c2[:], s2[:], r2[:])
                    d = dpool.tile([128, HW], bf16, name="d", tag="d")
                    nc.vector.tensor_sub(d[:], c1[:], c2[:])
                    dhalves.append(d)

                for j in range(4):  # w2 for this channel: 4 x [128, 2048]
                    wt = wpool.tile([128, 2048], bf16, name="w", tag="w")
                    col0 = (c * 8 + 2 * j) * 1024
                    nc.sync.dma_start(wt[:], w2_ap[:, col0:col0 + 2048])
                    for tt in range(2):
                        t = 2 * j + tt
                        d = dhalves[t // 4]
                        mm_block(d, (t % 4) * B_SH, wt, tt * 1024,
                                 start=False, stop=(c == 7 and t == 7))

            # ---- evacuate PSUM -> SBUF -> DRAM ----
            for bt in range(4):
                for oc in range(2):
                    ob = opool.tile([128, 512], f32, name="ob", tag="ob")
                    nc.scalar.copy(ob[:], psum[(bt, oc)][:, :])
                    nc.sync.dma_start(
                        out_ap[bt * 128:(bt + 1) * 128,
                               oc * 512:(oc + 1) * 512], ob[:])
    return nc


def _prep_weights(base_weight, spline_weight, spline_scaler):
    bf16 = ml_dtypes.bfloat16
    # wb[p, t*1024+o] = base_weight[o, t*128+p]
    wb = np.ascontiguousarray(
        base_weight.T.reshape(8, 128, 1024).transpose(1, 0, 2)
        .reshape(128, 8 * 1024)).astype(bf16)
    # eff_w[o,i,c] -> w2[p, (c*8+t)*1024 + o] = eff_w[o, t*128+p, c] / 6
    eff = (spline_weight * spline_scaler[..., None]) / 6.0   # (O, I, C)
    # -> (C, I, O) -> (C, T, P, O) -> (P, C, T, O)
    w2 = np.ascontiguousarray(
        eff.transpose(2, 1, 0).reshape(8, 8, 128, 1024).transpose(2, 0, 1, 3)
        .reshape(128, 64 * 1024)).astype(bf16)
    return wb, w2


def kernel(x, base_weight, spline_weight, spline_scaler, grid):
    from concourse.bass_utils import run_bass_kernel_spmd

    x = np.asarray(x, dtype=np.float32)
    base_weight = np.asarray(base_weight, dtype=np.float32)
    spline_weight = np.asarray(spline_weight, dtype=np.float32)
    spline_scaler = np.asarray(spline_scaler, dtype=np.float32)

    if "nc" not in _CACHE:
        _CACHE["nc"] = _build_program()
    nc = _CACHE["nc"]

    wb, w2 = _prep_weights(base_weight, spline_weight, spline_scaler)

    in_maps = []
    for r in range(N_CORES):
        xs = x[r * B_SH:(r + 1) * B_SH]  # (512, 1024)
        xt = np.ascontiguousarray(
            xs.T.reshape(8, 128, B_SH).transpose(1, 0, 2).reshape(128, 8 * B_SH))
        in_maps.append({"xt": xt, "wb": wb, "w2": w2})

    res = run_bass_kernel_spmd(nc, in_maps, core_ids=list(range(N_CORES)))
    out = np.concatenate([res.results[r]["out"] for r in range(N_CORES)], axis=0)
    return out.astype(np.float32)


# revision 7
# speedup vs baseline: 1.0448x; 1.0448x over previous
"""Trainium2 Bass kernel for a KAN (Kolmogorov-Arnold) layer.

Computation (see reference):
  out = silu(x) @ base_weight.T + bspline_basis(x).reshape(B,-1) @ (spline_weight*scaler).reshape(O,-1).T

Key ideas:
  * Data-parallel: batch 4096 is split across 8 NeuronCores (512 rows each);
    weights are replicated. No inter-core communication.
  * The cubic B-spline basis over the uniform grid (knots -2.2 + 0.4*j) has a
    closed form per output channel c (c = 0..7):
        s = (x + 2.2) / 0.4,   v = 2 - |s - (c+2)|
        6 * basis_c = relu(v)^3 - 4 * relu(v-1)^3
    (truncated-power representation of the cardinal cubic B-spline; the 1/6 is
    folded into the spline weights on the host).
  * Both matmuls run in bf16 on the tensor engine, accumulating fp32 into the
    same PSUM tiles: out[b,o] = sum_k silu_T[k,b] * WbT[k,o]  (k = 1024)
                              + sum_k  d_T[k,b]  * W2T[k,o]   (k = 8192)
    with k (contraction) on partitions, batch on PSUM partitions.
  * Per-core layouts are prepared on the host so every DMA is contiguous.
  * x is loaded in 4 chunks so silu + the base matmuls start early; the last
    spline channel runs psum-tile-major so evacuation overlaps its matmuls.
"""

import numpy as np
import ml_dtypes

N_CORES = 8
B_FULL = 4096
B_SH = B_FULL // N_CORES  # 512
IN_F = 1024
OUT_F = 1024
N_COEF = 8
GRID_T0 = -2.2  # first knot
GRID_H = 0.4    # knot spacing
C4 = 4.0 ** (1.0 / 3.0)

_CACHE = {}


def _build_program():
    import concourse.bass as bass
    import concourse.tile as tile
    from concourse import mybir
    from concourse.vector_clock import ScopedClock

    f32 = mybir.dt.float32
    bf16 = mybir.dt.bfloat16
    AF = mybir.ActivationFunctionType

    class SplitWaitTileContext(tile.TileContext):
        """The pinned walrus build only accepts a single sem-wait per
        instruction; hoist excess waits onto injected same-engine NoOps
        placed immediately before the over-subscribed instruction."""

        def _split_excess_waits(self):
            nc = self.nc
            k = 0
            for func in nc.m.functions:
                for bb in func.blocks:
                    il = bb.instructions
                    i = 0
                    while i < len(il):
                        inst = il[i]
                        si = inst.sync_info
                        if si is not None and si.on_wait and len(si.on_wait) > 1:
                            extra = list(si.on_wait)[1:]
                            del si.on_wait[1:]
                            for w in extra:
                                nop = mybir.InstNoOp(
                                    name=f"wsplit-{k}",
                                    engine=inst.engine,
                                    bass_nofuse=True,
                                    sync_info=mybir.SyncInfo(
                                        on_wait=[w], on_update=[]),
                                )
                                k += 1
                                nc.register_instruction(nop)
                                il.insert(i, nop)
                                i += 1
                        i += 1

        def _drain_and_barrier(self, tick_clock, wait_clock):
            nc = self.nc
            drain_inst = nc.sync.drain()
            wait_clock.add_sem_waits(
                drain_inst.ins, ScopedClock({None: tick_clock.global_clock})
            )
            self._split_excess_waits()
            nc.all_engine_barrier()
            assert self.sems is not None
            popped = nc._tile_sem_poison_stack.pop()
            assert popped is self._sem_poison
            nc.clear_and_free_semaphores(list(self.sems.allocated().values()))
            nc.all_engine_barrier()

    nc = bass.Bass("TRN2", target_bir_lowering=False, debug=False,
                   num_devices=N_CORES)

    # Host-prepared layouts (per core):
    #  xt [128, 4096] f32 : xt[p, t*512+b] = x_shard[b, t*128+p]
    #  wb [128, 8192] bf16: wb[p, t*1024+o] = base_weight[o, t*128+p]
    #  w2 [128, 65536] bf16: w2[p, (c*8+t)*1024+o] = eff_w[o, t*128+p, c]/6
    xt_ap = nc.dram_tensor("xt", [128, 8 * B_SH], f32, kind="ExternalInput").ap()
    wb_ap = nc.dram_tensor("wb", [128, 8 * 1024], bf16, kind="ExternalInput").ap()
    w2_ap = nc.dram_tensor("w2", [128, 64 * 1024], bf16, kind="ExternalInput").ap()
    out_ap = nc.dram_tensor("out", [B_SH, OUT_F], f32, kind="ExternalOutput").ap()

    HW = 4 * B_SH  # 2048: elementwise chunk width (half of the 4096 free dim)

    # activation bias values, by column of the bias tile
    BIAS_COLS = [2.0, C4] + [3.5 - c for c in range(8)]

    with SplitWaitTileContext(nc) as tc:
        import contextlib
        ctx = contextlib.ExitStack()
        with ctx:
            io_pool = ctx.enter_context(tc.tile_pool(name="io", bufs=1))
            wpool = ctx.enter_context(tc.tile_pool(name="w", bufs=6))
            apool = ctx.enter_context(tc.tile_pool(name="a", bufs=2))
            tpool = ctx.enter_context(tc.tile_pool(name="t", bufs=2))
            dpool = ctx.enter_context(tc.tile_pool(name="d", bufs=6))
            opool = ctx.enter_context(tc.tile_pool(name="o", bufs=4))
            psum_pool = ctx.enter_context(
                tc.tile_pool(name="ps", bufs=1, space="PSUM"))

            # bias constants for activations, Tile-tracked (no extra barrier)
            bias_t = io_pool.tile([128, len(BIAS_COLS)], f32, name="bias",
                                  tag="bias")
            for k, val in enumerate(BIAS_COLS):
                nc.gpsimd.memset(bias_t[:, k:k + 1], val)
            B_R1 = bias_t[:, 0:1]
            B_R2 = bias_t[:, 1:2]

            def babs(c):
                return bias_t[:, 2 + c:3 + c]

            # ---- x load in 4 chunks; silu per chunk ----
            xt = io_pool.tile([128, 8 * B_SH], f32, name="xt", tag="xt")
            silu_t = io_pool.tile([128, 8 * B_SH], bf16, name="silu",
                                  tag="silu")
            for q in range(4):
                qs = slice(q * 1024, (q + 1) * 1024)
                nc.sync.dma_start(xt[:, qs], xt_ap[:, qs])
                nc.scalar.activation(silu_t[:, qs], xt[:, qs], AF.Silu)

            # ---- PSUM output tiles: (bt, oc) -> [128 b, 512 o] ----
            psum = {}
            for bt in range(4):
                for oc in range(2):
                    psum[(bt, oc)] = psum_pool.tile(
                        [128, 512], f32, name=f"ps{bt}{oc}", tag=f"ps{bt}{oc}")

            def mm(bt, oc, lhs_tile, lhs_col0, w_tile, w_col0, start, stop):
                nc.tensor.matmul(
                    psum[(bt, oc)][:, :],
                    lhs_tile[:, lhs_col0 + bt * 128: lhs_col0 + bt * 128 + 128],
                    w_tile[:, w_col0 + oc * 512: w_col0 + oc * 512 + 512],
                    start=start, stop=stop,
                )

            def mm_block(lhs_tile, lhs_col0, w_tile, w_col0, start, stop):
                for bt in range(4):
                    for oc in range(2):
                        mm(bt, oc, lhs_tile, lhs_col0, w_tile, w_col0,
                           start, stop)

            # ---- base matmuls: 8 K-tiles over in_features ----
            for j in range(4):  # wb DMA'd as 4 x [128, 2048] (2 K-tiles each)
                wt = wpool.tile([128, 2048], bf16, name="w", tag="w")
                nc.sync.dma_start(wt[:], wb_ap[:, j * 2048:(j + 1) * 2048])
                for tt in range(2):
                    t = 2 * j + tt
                    mm_block(silu_t, t * B_SH, wt, tt * 1024,
                             start=(t == 0), stop=False)

            # ---- spline channels ----
            def elementwise(c):
                dhalves = []
                for h in range(2):  # 2 chunks of [128, 2048]
                    sl = slice(h * HW, (h + 1) * HW)
                    a = apool.tile([128, HW], f32, name="a", tag="a")
                    nc.scalar.activation(a[:], xt[:, sl], AF.Abs,
                                         bias=babs(c), scale=1.0 / GRID_H)
                    r1 = tpool.tile([128, HW], bf16, name="r1", tag="r1")
                    nc.scalar.activation(r1[:], a[:], AF.Relu, bias=B_R1,
                                         scale=-1.0)
                    r2 = tpool.tile([128, HW], bf16, name="r2", tag="r2")
                    nc.scalar.activation(r2[:], a[:], AF.Relu, bias=B_R2,
                                         scale=-C4)
                    s1 = tpool.tile([128, HW], bf16, name="s1", tag="s1")
                    nc.vector.tensor_mul(s1[:], r1[:], r1[:])
                    s2 = tpool.tile([128, HW], bf16, name="s2", tag="s2")
                    nc.vector.tensor_mul(s2[:], r2[:], r2[:])
                    c1 = tpool.tile([128, HW], bf16, name="c1", tag="c1")
                    nc.vector.tensor_mul(c1[:], s1[:], r1[:])
                    c2 = tpool.tile([128, HW], bf16, name="c2", tag="c2")
                    nc.vector.tensor_mul(c2[:], s2[:], r2[:])
                    d = dpool.tile([128, HW], bf16, name="d", tag="d")
                    nc.vector.tensor_sub(d[:], c1[:], c2[:])
                    dhalves.append(d)
                return dhalves

            for c in range(7):
                dhalves = elementwise(c)
                for j in range(4):  # w2 for this channel: 4 x [128, 2048]
                    wt = wpool.tile([128, 2048], bf16, name="w", tag="w")
                    col0 = (c * 8 + 2 * j) * 1024
                    nc.sync.dma_start(wt[:], w2_ap[:, col0:col0 + 2048])
                    for tt in range(2):
                        t = 2 * j + tt
                        mm_block(dhalves[t // 4], (t % 4) * B_SH, wt, tt * 1024,
                                 start=False, stop=False)

            # last channel: psum-tile-major so evacuation overlaps matmuls
            c = 7
            dhalves = elementwise(c)
            wts = []
            for j in range(4):
                wt = wpool.tile([128, 2048], bf16, name="w", tag="w")
                col0 = (c * 8 + 2 * j) * 1024
                nc.sync.dma_start(wt[:], w2_ap[:, col0:col0 + 2048])
                wts.append(wt)
            for bt in range(4):
                for oc in range(2):
                    for t in range(8):
                        mm(bt, oc, dhalves[t // 4], (t % 4) * B_SH,
                           wts[t // 2], (t % 2) * 1024,
                           start=False, stop=(t == 7))
                    ob = opool.tile([128, 512], f32, name="ob", tag="ob")
                    nc.scalar.copy(ob[:], psum[(bt, oc)][:, :])
                    nc.sync.dma_start(
                        out_ap[bt * 128:(bt + 1) * 128,
                               oc * 512:(oc + 1) * 512], ob[:])
    return nc


def _prep_weights(base_weight, spline_weight, spline_scaler):
    bf16 = ml_dtypes.bfloat16
    # wb[p, t*1024+o] = base_weight[o, t*128+p]
    wb = np.ascontiguousarray(
        base_weight.T.reshape(8, 128, 1024).transpose(1, 0, 2)
        .reshape(128, 8 * 1024)).astype(bf16)
    # eff_w[o,i,c] -> w2[p, (c*8+t)*1024 + o] = eff_w[o, t*128+p, c] / 6
    eff = (spline_weight * spline_scaler[..., None]) / 6.0   # (O, I, C)
    # -> (C, I, O) -> (C, T, P, O) -> (P, C, T, O)
    w2 = np.ascontiguousarray(
        eff.transpose(2, 1, 0).reshape(8, 8, 128, 1024).transpose(2, 0, 1, 3)
        .reshape(128, 64 * 1024)).astype(bf16)
    return wb, w2


def kernel(x, base_weight, spline_weight, spline_scaler, grid):
    from concourse.bass_utils import run_bass_kernel_spmd

    x = np.asarray(x, dtype=np.float32)
    base_weight = np.asarray(base_weight, dtype=np.float32)
    spline_weight = np.asarray(spline_weight, dtype=np.float32)
    spline_scaler = np.asarray(spline_scaler, dtype=np.float32)

    if "nc" not in _CACHE:
        _CACHE["nc"] = _build_program()
    nc = _CACHE["nc"]

    wb, w2 = _prep_weights(base_weight, spline_weight, spline_scaler)

    in_maps = []
    for r in range(N_CORES):
        xs = x[r * B_SH:(r + 1) * B_SH]  # (512, 1024)
        xt = np.ascontiguousarray(
            xs.T.reshape(8, 128, B_SH).transpose(1, 0, 2).reshape(128, 8 * B_SH))
        in_maps.append({"xt": xt, "wb": wb, "w2": w2})

    res = run_bass_kernel_spmd(nc, in_maps, core_ids=list(range(N_CORES)))
    out = np.concatenate([res.results[r]["out"] for r in range(N_CORES)], axis=0)
    return out.astype(np.float32)


# revision 9
# speedup vs baseline: 1.0749x; 1.0288x over previous
"""Trainium2 Bass kernel for a KAN (Kolmogorov-Arnold) layer.

Computation (see reference):
  out = silu(x) @ base_weight.T + bspline_basis(x).reshape(B,-1) @ (spline_weight*scaler).reshape(O,-1).T

Key ideas:
  * Data-parallel: batch 4096 is split across 8 NeuronCores (512 rows each);
    weights are replicated. No inter-core communication.
  * The cubic B-spline basis over the uniform grid (knots -2.2 + 0.4*j) has a
    closed form per output channel c (c = 0..7):
        s = (x + 2.2) / 0.4,   v = 2 - |s - (c+2)|
        6 * basis_c = relu(v)^3 - 4 * relu(v-1)^3
    (truncated-power representation of the cardinal cubic B-spline; the 1/6 is
    folded into the spline weights on the host).
  * Both matmuls run in bf16 on the tensor engine, accumulating fp32 into the
    same PSUM tiles: out[b,o] = sum_k silu_T[k,b] * WbT[k,o]  (k = 1024)
                              + sum_k  d_T[k,b]  * W2T[k,o]   (k = 8192)
    with k (contraction) on partitions, batch on PSUM partitions.
  * Per-core layouts are prepared on the host so every DMA is contiguous.
  * x is loaded in 4 chunks so silu + the base matmuls start early; the last
    spline channel runs psum-tile-major so evacuation overlaps its matmuls.
"""

import numpy as np
import ml_dtypes

N_CORES = 8
B_FULL = 4096
B_SH = B_FULL // N_CORES  # 512
IN_F = 1024
OUT_F = 1024
N_COEF = 8
GRID_T0 = -2.2  # first knot
GRID_H = 0.4    # knot spacing
C4 = 4.0 ** (1.0 / 3.0)

_CACHE = {}


def _build_program():
    import concourse.bass as bass
    import concourse.tile as tile
    from concourse import mybir
    from concourse.vector_clock import ScopedClock

    f32 = mybir.dt.float32
    bf16 = mybir.dt.bfloat16
    AF = mybir.ActivationFunctionType

    class SplitWaitTileContext(tile.TileContext):
        """The pinned walrus build only accepts a single sem-wait per
        instruction; hoist excess waits onto injected same-engine NoOps
        placed immediately before the over-subscribed instruction."""

        def _split_excess_waits(self):
            nc = self.nc
            k = 0
            for func in nc.m.functions:
                for bb in func.blocks:
                    il = bb.instructions
                    i = 0
                    while i < len(il):
                        inst = il[i]
                        si = inst.sync_info
                        if si is not None and si.on_wait and len(si.on_wait) > 1:
                            extra = list(si.on_wait)[1:]
                            del si.on_wait[1:]
                            for w in extra:
                                nop = mybir.InstNoOp(
                                    name=f"wsplit-{k}",
                                    engine=inst.engine,
                                    bass_nofuse=True,
                                    sync_info=mybir.SyncInfo(
                                        on_wait=[w], on_update=[]),
                                )
                                k += 1
                                nc.register_instruction(nop)
                                il.insert(i, nop)
                                i += 1
                        i += 1

        def _drain_and_barrier(self, tick_clock, wait_clock):
            nc = self.nc
            drain_inst = nc.sync.drain()
            wait_clock.add_sem_waits(
                drain_inst.ins, ScopedClock({None: tick_clock.global_clock})
            )
            self._split_excess_waits()
            nc.all_engine_barrier()
            assert self.sems is not None
            popped = nc._tile_sem_poison_stack.pop()
            assert popped is self._sem_poison
            nc.clear_and_free_semaphores(list(self.sems.allocated().values()))
            nc.all_engine_barrier()

    nc = bass.Bass("TRN2", target_bir_lowering=False, debug=False,
                   num_devices=N_CORES)

    # Host-prepared layouts (per core):
    #  xt [128, 4096] f32 : xt[p, t*512+b] = x_shard[b, t*128+p]
    #  wb [128, 8192] bf16: wb[p, t*1024+o] = base_weight[o, t*128+p]
    #  w2 [128, 65536] bf16: w2[p, (c*8+t)*1024+o] = eff_w[o, t*128+p, c]/6
    xt_ap = nc.dram_tensor("xt", [128, 8 * B_SH], f32, kind="ExternalInput").ap()
    wb_ap = nc.dram_tensor("wb", [128, 8 * 1024], bf16, kind="ExternalInput").ap()
    w2_ap = nc.dram_tensor("w2", [128, 64 * 1024], bf16, kind="ExternalInput").ap()
    out_ap = nc.dram_tensor("out", [B_SH, OUT_F], f32, kind="ExternalOutput").ap()

    HW = 4 * B_SH  # 2048: elementwise chunk width (half of the 4096 free dim)

    # activation bias values, by column of the bias tile
    BIAS_COLS = [2.0, C4] + [3.5 - c for c in range(8)]

    with SplitWaitTileContext(nc) as tc:
        import contextlib
        ctx = contextlib.ExitStack()
        with ctx:
            io_pool = ctx.enter_context(tc.tile_pool(name="io", bufs=1))
            wpool = ctx.enter_context(tc.tile_pool(name="w", bufs=8))
            apool = ctx.enter_context(tc.tile_pool(name="a", bufs=3))
            tpool = ctx.enter_context(tc.tile_pool(name="t", bufs=3))
            dpool = ctx.enter_context(tc.tile_pool(name="d", bufs=16))
            opool = ctx.enter_context(tc.tile_pool(name="o", bufs=4))
            psum_pool = ctx.enter_context(
                tc.tile_pool(name="ps", bufs=1, space="PSUM"))

            # bias constants for activations, Tile-tracked (no extra barrier)
            bias_t = io_pool.tile([128, len(BIAS_COLS)], f32, name="bias",
                                  tag="bias")
            for k, val in enumerate(BIAS_COLS):
                nc.gpsimd.memset(bias_t[:, k:k + 1], val)
            B_R1 = bias_t[:, 0:1]
            B_R2 = bias_t[:, 1:2]

            def babs(c):
                return bias_t[:, 2 + c:3 + c]

            # ---- PSUM output tiles: (bt, oc) -> [128 b, 512 o] ----
            psum = {}
            for bt in range(4):
                for oc in range(2):
                    psum[(bt, oc)] = psum_pool.tile(
                        [128, 512], f32, name=f"ps{bt}{oc}", tag=f"ps{bt}{oc}")

            def mm(bt, oc, lhs_tile, lhs_col0, w_tile, w_col0, start, stop):
                nc.tensor.matmul(
                    psum[(bt, oc)][:, :],
                    lhs_tile[:, lhs_col0 + bt * 128: lhs_col0 + bt * 128 + 128],
                    w_tile[:, w_col0 + oc * 512: w_col0 + oc * 512 + 512],
                    start=start, stop=stop,
                )

            def mm_block(lhs_tile, lhs_col0, w_tile, w_col0, start, stop):
                for bt in range(4):
                    for oc in range(2):
                        mm(bt, oc, lhs_tile, lhs_col0, w_tile, w_col0,
                           start, stop)

            # ---- x load in 4 separate chunk tiles (so deps are per-chunk);
            #      silu per chunk; base matmuls follow each chunk ----
            xts, silus = [], []
            for q in range(4):
                qs = slice(q * 1024, (q + 1) * 1024)
                xtq = io_pool.tile([128, 1024], f32, name=f"xt{q}",
                                   tag=f"xt{q}")
                nc.sync.dma_start(xtq[:], xt_ap[:, qs])
                siq = io_pool.tile([128, 1024], bf16, name=f"silu{q}",
                                   tag=f"silu{q}")
                nc.scalar.activation(siq[:], xtq[:], AF.Silu)
                xts.append(xtq)
                silus.append(siq)
                # base weights for K-tiles 2q, 2q+1 + their matmuls
                wt = wpool.tile([128, 2048], bf16, name="w", tag="w")
                nc.sync.dma_start(wt[:], wb_ap[:, q * 2048:(q + 1) * 2048])
                for tt in range(2):
                    t = 2 * q + tt
                    mm_block(siq, tt * B_SH, wt, tt * 1024,
                             start=(t == 0), stop=False)

            # ---- spline channels (elementwise per quarter chunk) ----
            def elementwise(c):
                dquarts = []
                for q in range(4):  # 4 chunks of [128, 1024]
                    a = apool.tile([128, 1024], f32, name="a", tag="a")
                    nc.scalar.activation(a[:], xts[q][:], AF.Abs,
                                         bias=babs(c), scale=1.0 / GRID_H)
                    r1 = tpool.tile([128, 1024], bf16, name="r1", tag="r1")
                    nc.scalar.activation(r1[:], a[:], AF.Relu, bias=B_R1,
                                         scale=-1.0)
                    r2 = tpool.tile([128, 1024], bf16, name="r2", tag="r2")
                    nc.scalar.activation(r2[:], a[:], AF.Relu, bias=B_R2,
                                         scale=-C4)
                    s1 = tpool.tile([128, 1024], bf16, name="s1", tag="s1")
                    nc.vector.tensor_mul(s1[:], r1[:], r1[:])
                    s2 = tpool.tile([128, 1024], bf16, name="s2", tag="s2")
                    nc.vector.tensor_mul(s2[:], r2[:], r2[:])
                    c1 = tpool.tile([128, 1024], bf16, name="c1", tag="c1")
                    nc.vector.tensor_mul(c1[:], s1[:], r1[:])
                    c2 = tpool.tile([128, 1024], bf16, name="c2", tag="c2")
                    nc.vector.tensor_mul(c2[:], s2[:], r2[:])
                    d = dpool.tile([128, 1024], bf16, name="d", tag="d")
                    nc.vector.tensor_sub(d[:], c1[:], c2[:])
                    dquarts.append(d)
                return dquarts

            for c in range(7):
                dq = elementwise(c)
                for j in range(4):  # w2 for this channel: 4 x [128, 2048]
                    wt = wpool.tile([128, 2048], bf16, name="w", tag="w")
                    col0 = (c * 8 + 2 * j) * 1024
                    nc.sync.dma_start(wt[:], w2_ap[:, col0:col0 + 2048])
                    for tt in range(2):
                        t = 2 * j + tt
                        mm_block(dq[t // 2], (t % 2) * B_SH, wt, tt * 1024,
                                 start=False, stop=False)

            # last channel: psum-tile-major so evacuation overlaps matmuls
            c = 7
            dq = elementwise(c)
            wts = []
            for j in range(4):
                wt = wpool.tile([128, 2048], bf16, name="w", tag="w")
                col0 = (c * 8 + 2 * j) * 1024
                nc.sync.dma_start(wt[:], w2_ap[:, col0:col0 + 2048])
                wts.append(wt)
            for bt in range(4):
                for oc in range(2):
                    for t in range(8):
                        mm(bt, oc, dq[t // 2], (t % 2) * B_SH,
                           wts[t // 2], (t % 2) * 1024,
                           start=False, stop=(t == 7))
                    ob = opool.tile([128, 512], f32, name="ob", tag="ob")
                    nc.scalar.copy(ob[:], psum[(bt, oc)][:, :])
                    nc.sync.dma_start(
                        out_ap[bt * 128:(bt + 1) * 128,
                               oc * 512:(oc + 1) * 512], ob[:])
    return nc


def _prep_weights(base_weight, spline_weight, spline_scaler):
    bf16 = ml_dtypes.bfloat16
    # wb[p, t*1024+o] = base_weight[o, t*128+p]
    wb = np.ascontiguousarray(
        base_weight.T.reshape(8, 128, 1024).transpose(1, 0, 2)
        .reshape(128, 8 * 1024)).astype(bf16)
    # eff_w[o,i,c] -> w2[p, (c*8+t)*1024 + o] = eff_w[o, t*128+p, c] / 6
    eff = (spline_weight * spline_scaler[..., None]) / 6.0   # (O, I, C)
    # -> (C, I, O) -> (C, T, P, O) -> (P, C, T, O)
    w2 = np.ascontiguousarray(
        eff.transpose(2, 1, 0).reshape(8, 8, 128, 1024).transpose(2, 0, 1, 3)
        .reshape(128, 64 * 1024)).astype(bf16)
    return wb, w2


def kernel(x, base_weight, spline_weight, spline_scaler, grid):
    from concourse.bass_utils import run_bass_kernel_spmd

    x = np.asarray(x, dtype=np.float32)
    base_weight = np.asarray(base_weight, dtype=np.float32)
    spline_weight = np.asarray(spline_weight, dtype=np.float32)
    spline_scaler = np.asarray(spline_scaler, dtype=np.float32)

    if "nc" not in _CACHE:
        _CACHE["nc"] = _build_program()
    nc = _CACHE["nc"]

    wb, w2 = _prep_weights(base_weight, spline_weight, spline_scaler)

    in_maps = []
    for r in range(N_CORES):
        xs = x[r * B_SH:(r + 1) * B_SH]  # (512, 1024)
        xt = np.ascontiguousarray(
            xs.T.reshape(8, 128, B_SH).transpose(1, 0, 2).reshape(128, 8 * B_SH))
        in_maps.append({"xt": xt, "wb": wb, "w2": w2})

    res = run_bass_kernel_spmd(nc, in_maps, core_ids=list(range(N_CORES)))
    out = np.concatenate([res.results[r]["out"] for r in range(N_CORES)], axis=0)
    return out.astype(np.float32)
